# revision 55
# baseline (speedup 1.0000x reference)
"""Causal self-attention (QK-RMSNorm + RoPE) on 8 Trainium2 NeuronCores.

Problem: x[2,2048,2048], Wq/Wk/Wv/Wo [2048,2048], 16 heads, head_dim 128.

Sharding: core c handles batch b=c//4 and head group g=c%4 (4 heads,
model cols [512g:512g+512)).

Structure (v14):
  * QKV projection and attention are INTERLEAVED: attention chunk ic
    (dependency-limited: exp on ACT, masks/den on DVE) is spliced between
    the dense projection ib-passes of chunk ic+1, so the PE stays
    saturated and the HAM clock gate never re-throttles.  Q/K are
    RMS-normed + RoPE'd in row layout and PE-transposed into persistent
    SBUF tiles (no DRAM round trip).
  * Transposed scores: eT = exp(scale*kT_blk.T @ qT_chunk - 1); the causal
    mask is folded into the score accumulation as a second matmul
    (-30000*I @ above_diag_pattern), so masked entries underflow to 0 in
    the exp and no elementwise mask sits in the AV critical path.
  * Softmax denominator: DVE accumulates et tiles into [128,512] f32; a
    small matmul with (1/32) "ones" folds it across partitions, and a K=1
    matmul broadcasts the f16 reciprocal (32/den fits f16 normal range).
    The chain is spliced across the next projection pass boundaries so it
    never stalls the PE.  The 1/32 is divided back out at o_proj eviction.
  * o_proj: each core multiplies its yT head shard against its ROW slice
    of Wo.T, producing a full [2048 out, 2048 pos] f16 partial that the
    host sums across the 4 head groups.  No collective at all.  Runs as a
    dense tail phase with 4 PSUM banks.

Matmuls run with f16 operands (full PE rate).
"""

import math
from contextlib import ExitStack

import numpy as np

import concourse.bass as bass
import concourse.bacc as bacc
import concourse.tile as tile
from concourse import mybir
from concourse.bass_utils import run_bass_kernel_spmd
from concourse.masks import make_identity

P = 128
D = 2048
S = 2048
HD = 128              # head dim
NHL = 4               # heads per core
GW = NHL * HD         # 512, per-core width of head group
CT = D // P           # 16 contraction tiles
ICH = 4               # i-chunks of 512 positions
NCORES = 8
F32 = mybir.dt.float32
F16 = mybir.dt.float16
BF16 = mybir.dt.bfloat16
SCALE = 1.0 / math.sqrt(HD)
EPS = 1.1920928955078125e-07
# den spans roughly [1e-3, 1e5] over the causal rows; 32/den centers the
# reciprocal inside f16 normal range so the broadcast matmul can run in f16
DEN_SCALE = 32.0

_program_cache = {}


def build_program():
    if "nc" in _program_cache:
        return _program_cache["nc"]

    nc = bacc.Bacc("TRN2", target_bir_lowering=False, debug=False, num_devices=NCORES)

    xt_in = nc.dram_tensor("xt", [D, S], F16, kind="ExternalInput")
    wq_in = nc.dram_tensor("wq", [D, GW], F16, kind="ExternalInput")
    wk_in = nc.dram_tensor("wk", [D, GW], F16, kind="ExternalInput")
    wv_in = nc.dram_tensor("wv", [D, GW], F16, kind="ExternalInput")
    wo_in = nc.dram_tensor("wo", [GW, D], F16, kind="ExternalInput")
    cos_in = nc.dram_tensor("cos", [S, HD // 2], F16, kind="ExternalInput")
    sin_in = nc.dram_tensor("sin", [S, HD // 2], F16, kind="ExternalInput")
    mask_in = nc.dram_tensor("maskt", [4, P, 512], F16, kind="ExternalInput")
    yt_out = nc.dram_tensor("yt_out", [D, S], F16, kind="ExternalOutput")

    with tile.TileContext(nc) as tc:
        with ExitStack() as ctx:
            const = ctx.enter_context(tc.tile_pool(name="const", bufs=1))

            ident = const.tile([P, P], F16, name="ident")
            make_identity(nc, ident)
            negI = const.tile([P, P], F16, name="negI")
            nc.scalar.activation(
                negI[:], ident[:], mybir.ActivationFunctionType.Copy,
                bias=0.0, scale=-30000.0,
            )
            eps_t = const.tile([P, 1], F32, name="eps_t")
            nc.vector.memset(eps_t[:], EPS)
            neg1_t = const.tile([P, 1], F32, name="neg1_t")
            nc.vector.memset(neg1_t[:], -1.0)
            ones_f = const.tile([P, P], F32, name="ones_f")
            nc.vector.memset(ones_f[:], 1.0)
            onessc = const.tile([P, 2], F32, name="onessc")
            nc.vector.memset(onessc[:], 1.0 / DEN_SCALE)
            ones2 = const.tile([P, 2], BF16, name="ones2")
            nc.scalar.copy(ones2[:], onessc[:])
            ones_row = const.tile([1, P], F16, name="ones_row")
            nc.scalar.copy(ones_row[:], ones_f[0:1, :])
            invsc_t = const.tile([P, 1], F32, name="invsc_t")
            nc.vector.memset(invsc_t[:], 1.0 / DEN_SCALE)

            cos_sb = const.tile([P, CT, HD // 2], F16, name="cos_sb")
            nc.sync.dma_start(out=cos_sb[:], in_=cos_in.ap().rearrange("(a p) f -> p a f", p=P))
            sin_sb = const.tile([P, CT, HD // 2], F16, name="sin_sb")
            nc.sync.dma_start(out=sin_sb[:], in_=sin_in.ap().rearrange("(a p) f -> p a f", p=P))
            mask_sb = const.tile([P, 4, 512], F16, name="mask_sb")
            nc.sync.dma_start(out=mask_sb[:], in_=mask_in.ap().rearrange("t p f -> p t f"))

            # persistent tensors (live through the whole kernel)
            persist = ctx.enter_context(tc.tile_pool(name="persist", bufs=1))
            wq_sb = persist.tile([P, CT, GW], F16, name="wq_sb")
            wk_sb = persist.tile([P, CT, GW], F16, name="wk_sb")
            wv_sb = persist.tile([P, CT, GW], F16, name="wv_sb")
            wo_sb = persist.tile([P, NHL, D], F16, name="wo_sb")
            qt_sb = persist.tile([P, NHL, S], F16, name="qt_sb")
            kt_sb = persist.tile([P, NHL, S], F16, name="kt_sb")
            v_sb = persist.tile([P, CT, GW], F16, name="v_sb")

            # normalized attention outputs for all 4 chunks (consumed by the
            # o_proj tail phase)
            ytn_pool = ctx.enter_context(tc.tile_pool(name="ytn_pool", bufs=4))
            ytn_tiles = {}

            # ---------------- merged phase: QKV projection + attention ------
            with ExitStack() as pha:
                xt_pool = pha.enter_context(tc.tile_pool(name="xt_pool", bufs=2))
                proj_ps = pha.enter_context(tc.tile_pool(name="proj_ps", bufs=2, space="PSUM"))
                tp_ps = pha.enter_context(tc.tile_pool(name="tp_ps", bufs=1, space="PSUM"))
                rope = pha.enter_context(tc.tile_pool(name="rope", bufs=3))
                stat = pha.enter_context(tc.tile_pool(name="stat", bufs=3))
                s_ps = pha.enter_context(tc.tile_pool(name="s_ps", bufs=3, space="PSUM"))
                acc_ps = pha.enter_context(tc.tile_pool(name="acc_ps", bufs=1, space="PSUM"))
                dbc_ps = pha.enter_context(tc.tile_pool(name="dbc_ps", bufs=1, space="PSUM"))
                et_pool = pha.enter_context(tc.tile_pool(name="et_pool", bufs=6))
                den_pool = pha.enter_context(tc.tile_pool(name="den_pool", bufs=2))
                bsmall = pha.enter_context(tc.tile_pool(name="bsmall", bufs=2))

                # dummy matmuls bridge the initial weight/x DMA wait so the
                # HAM clock gate is already released when real work arrives
                warm = dbc_ps.tile([P, 512], F32, name="warm", tag="dbc")
                NWARM = 220
                for i in range(NWARM):
                    nc.tensor.matmul(
                        warm[:, 0:P], ident[:], ident[:],
                        start=(i == 0), stop=(i == NWARM - 1),
                    )
                warm_out = const.tile([P, 1], F32, name="warm_out")
                nc.scalar.copy(warm_out[:], warm[:, 0:1])

                xt_tiles = {}

                def a_dmas(ch):
                    xt_ch = xt_pool.tile([P, CT, 512], F16, name=f"xt_ch{ch}", tag="xt")
                    xt_tiles[ch] = xt_ch
                    for ct in range(CT):
                        if ch == 0:
                            nc.sync.dma_start(out=wq_sb[:, ct, :], in_=wq_in[ct * P:(ct + 1) * P, :])
                        nc.sync.dma_start(
                            out=xt_ch[:, ct, :],
                            in_=xt_in[ct * P:(ct + 1) * P, ch * 512:(ch + 1) * 512],
                        )
                    if ch == 0:
                        for ct in range(CT):
                            nc.sync.dma_start(out=wk_sb[:, ct, :], in_=wk_in[ct * P:(ct + 1) * P, :])
                        for ct in range(CT):
                            nc.sync.dma_start(out=wv_sb[:, ct, :], in_=wv_in[ct * P:(ct + 1) * P, :])
                    if ch == 1:
                        for hb in range(NHL):
                            nc.sync.dma_start(
                                out=wo_sb[:, hb, :], in_=wo_in[hb * P:(hb + 1) * P, :]
                            )

                def norm_rope(ps_, ibg, mat):
                    qs = rope.tile([P, GW], F16, name=f"qs{mat}{ibg}", tag="qs")
                    nc.scalar.copy(qs[:], ps_[:])
                    sq = rope.tile([P, GW], F16, name=f"sq{mat}{ibg}", tag="sq")
                    nc.vector.tensor_mul(sq[:], qs[:], qs[:])
                    rstd = stat.tile([P, NHL], F32, name=f"rstd{mat}{ibg}", tag="rstd")
                    nc.vector.reduce_sum(
                        rstd[:],
                        sq[:].rearrange("p (h d) -> p h d", h=NHL),
                        axis=mybir.AxisListType.X,
                    )
                    nc.scalar.activation(
                        rstd[:], rstd[:],
                        mybir.ActivationFunctionType.Sqrt,
                        bias=eps_t[:], scale=1.0 / HD,
                    )
                    nc.vector.reciprocal(rstd[:], rstd[:])

                    q3 = qs[:].rearrange("p (h d) -> p h d", h=NHL)
                    qr = rope.tile([P, GW], F16, name=f"qr{mat}{ibg}", tag="qr")
                    qr3 = qr[:].rearrange("p (h d) -> p h d", h=NHL)
                    tmp = rope.tile([P, NHL, HD // 2], F16, name=f"tmp{mat}{ibg}", tag="tmp")
                    cosB = cos_sb[:, ibg:ibg + 1, :].broadcast_to((P, NHL, HD // 2))
                    sinB = sin_sb[:, ibg:ibg + 1, :].broadcast_to((P, NHL, HD // 2))
                    h1 = q3[:, :, 0:HD // 2]
                    h2 = q3[:, :, HD // 2:HD]
                    # r1 = q1*cos + q2*sin ; r2 = q2*cos - q1*sin
                    nc.vector.tensor_mul(qr3[:, :, 0:HD // 2], h1, cosB)
                    nc.vector.tensor_mul(tmp[:], h2, sinB)
                    nc.vector.tensor_add(qr3[:, :, 0:HD // 2], qr3[:, :, 0:HD // 2], tmp[:])
                    nc.vector.tensor_mul(qr3[:, :, HD // 2:HD], h2, cosB)
                    nc.vector.tensor_mul(tmp[:], h1, sinB)
                    nc.vector.tensor_sub(
                        qr3[:, :, HD // 2:HD], qr3[:, :, HD // 2:HD], tmp[:]
                    )
                    for h in range(NHL):
                        nc.vector.tensor_scalar_mul(
                            qr[:, h * HD:(h + 1) * HD],
                            qr[:, h * HD:(h + 1) * HD],
                            rstd[:, h:h + 1],
                        )
                    return qr

                pend_tp = []  # transposes deferred ~2 units behind the DVE chain

                def flush_tp():
                    qr, ibg, dst_t, mat = pend_tp.pop(0)
                    # all 4 heads transpose into one PSUM tile back-to-back,
                    # then a single strided copy writes the persistent tensor
                    tp = tp_ps.tile([P, NHL, P], F16, name=f"tp{mat}{ibg}", tag="tp")
                    for h in range(NHL):
                        nc.tensor.transpose(tp[:, h, :], qr[:, h * HD:(h + 1) * HD], ident[:])
                    nc.scalar.copy(dst_t[:, :, ibg * P:(ibg + 1) * P], tp[:])

                MATS = (("q", wq_sb, qt_sb), ("k", wk_sb, kt_sb), ("v", wv_sb, None))

                def a_ib_unit(ch, mi, ib, splice=None):
                    """One (matrix, i-block) projection pass: 16 dense matmuls
                    (+ spliced den-chain PE ops), then the norm/rope chain."""
                    mat, wsb, dst_t = MATS[mi]
                    ibg = ch * 4 + ib
                    xt_ch = xt_tiles[ch]
                    ps = proj_ps.tile([P, GW], F32, name=f"ps{mat}{ibg}", tag="proj")
                    for ct in range(CT):
                        nc.tensor.matmul(
                            ps[:],
                            xt_ch[:, ct, ib * P:(ib + 1) * P],
                            wsb[:, ct, :],
                            start=(ct == 0),
                            stop=(ct == CT - 1),
                        )
                    if splice is not None:
                        splice()
                    if mat == "v":
                        nc.scalar.copy(v_sb[:, ibg, :], ps[:])
                    else:
                        if len(pend_tp) >= 2:
                            flush_tp()
                        qr = norm_rope(ps, ibg, mat)
                        pend_tp.append((qr, ibg, dst_t, mat))

                # ---- attention pieces ----
                def bh(ic, h):
                    """One head's attention for i-chunk ic; returns the den
                    chain state (finished by spliced chain steps)."""
                    njb = 4 * ic + 4
                    ytn = ytn_tiles[ic]
                    ytp = acc_ps.tile([P, 512], F32, name=f"yt{h}_{ic}", tag="yt")
                    dacc = den_pool.tile([P, 512], F32, name=f"den{h}_{ic}", tag="den")
                    ets = {}

                    def score(jb):
                        t = jb - 4 * ic
                        sp = s_ps.tile([P, 512], F32, name=f"s{h}_{ic}_{jb}", tag="s")
                        nc.tensor.matmul(
                            sp[:],
                            kt_sb[:, h, jb * P:(jb + 1) * P],
                            qt_sb[:, h, ic * 512:(ic + 1) * 512],
                            start=True, stop=(t < 0),
                        )
                        if t >= 0:
                            nc.tensor.matmul(
                                sp[:], negI[:], mask_sb[:, t, :],
                                start=False, stop=True,
                            )
                        et = et_pool.tile([P, 512], F16, name=f"et{h}_{ic}_{jb}", tag="et")
                        nc.scalar.activation(
                            et[:], sp[:],
                            mybir.ActivationFunctionType.Exp,
                            bias=neg1_t[:], scale=SCALE,
                        )
                        ets[jb] = et

                    def consume(jb):
                        et = ets.pop(jb)
                        nc.tensor.matmul(
                            ytp[:],
                            v_sb[:, jb, h * HD:(h + 1) * HD],
                            et[:],
                            start=(jb == 0), stop=(jb == njb - 1),
                        )
                        if jb == 0:
                            nc.vector.tensor_copy(dacc[:], et[:])
                        else:
                            nc.vector.tensor_add(dacc[:], dacc[:], et[:])

                    for jb in range(min(3, njb)):
                        score(jb)
                    for jb in range(njb):
                        if jb + 3 < njb:
                            score(jb + 3)
                        consume(jb)

                    # cast for the den fold is issued now (ACT digests it
                    # while the next dense pass runs)
                    dacc_b = den_pool.tile([P, 512], BF16, name=f"daccb{h}_{ic}", tag="daccb")
                    nc.scalar.copy(dacc_b[:], dacc[:])
                    return {"ytp": ytp, "dacc_b": dacc_b, "ytn": ytn, "h": h, "ic": ic}

                def chain_fold(st):
                    # fold den across partitions; reciprocal lands in f16
                    dbc = dbc_ps.tile([P, 512], F32, name=f"dbc{st['h']}_{st['ic']}", tag="dbc")
                    st["dbc"] = dbc
                    nc.tensor.matmul(dbc[0:2, :], ones2[:], st["dacc_b"][:], start=True, stop=True)
                    rden = bsmall.tile([1, 512], F32, name=f"rd{st['h']}_{st['ic']}", tag="rden")
                    nc.vector.reciprocal(rden[:], dbc[0:1, :])
                    rden_r = bsmall.tile([1, 512], F16, name=f"rdr{st['h']}_{st['ic']}", tag="rdenr")
                    nc.scalar.copy(rden_r[:], rden[:])
                    st["rden_r"] = rden_r

                def chain_bc(st):
                    nc.tensor.matmul(st["dbc"][:], ones_row[:], st["rden_r"][:], start=True, stop=True)

                def chain_mul(st):
                    bc_sb = bsmall.tile([P, 512], F32, name=f"bcs{st['h']}_{st['ic']}", tag="bcs")
                    nc.vector.tensor_copy(bc_sb[:], st["dbc"][:])
                    nc.vector.tensor_mul(st["ytn"][:, st["h"], :], st["ytp"][:], bc_sb[:])

                # ---- schedule ----
                a_dmas(0)
                a_dmas(1)
                for mi in range(3):
                    for ib in range(4):
                        a_ib_unit(0, mi, ib)
                while pend_tp:
                    # chunk 0's last transposes must land before its
                    # attention reads qt/kt below
                    flush_tp()

                for ic in range(ICH - 1):
                    ch = ic + 1
                    if ch + 1 < ICH:
                        a_dmas(ch + 1)
                    ytn_tiles[ic] = ytn_pool.tile([P, NHL, 512], F16,
                                                  name=f"ytn{ic}", tag="ytn")
                    units = [(mi, ib) for mi in range(3) for ib in range(4)]
                    for h in range(NHL):
                        st = bh(ic, h)
                        for step, spl in ((0, chain_fold), (1, chain_bc), (2, chain_mul)):
                            mi, ib = units.pop(0)
                            a_ib_unit(ch, mi, ib,
                                      splice=(lambda s=st, f=spl: f(s)))
                    while pend_tp:
                        flush_tp()

                # last chunk's attention: o_proj blocks of the finished
                # chunks 0-2 serve as the dense PE filler (their PSUM bank is
                # borrowed from the den pool; evictions go through DVE since
                # ACT is saturated with this chunk's exps)
                done_obs = set()

                def oproj_unit(icq, ob, splice=None):
                    # borrows the (now idle) projection pool's PSUM banks
                    done_obs.add((icq, ob))
                    ytnq = ytn_tiles[icq]
                    yp = proj_ps.tile([P, 512], F32, name=f"ypf{icq}_{ob}", tag="proj")
                    for hh in range(NHL):
                        nc.tensor.matmul(
                            yp[:],
                            wo_sb[:, hh, ob * P:(ob + 1) * P],
                            ytnq[:, hh, :],
                            start=(hh == 0), stop=(hh == NHL - 1),
                        )
                    if splice is not None:
                        splice()
                    ysb = bsmall.tile([P, 512], F16, name=f"ysbf{icq}_{ob}", tag="ysbf")
                    nc.vector.tensor_scalar_mul(ysb[:], yp[:], invsc_t[:])
                    nc.scalar.dma_start(
                        out=yt_out[ob * P:(ob + 1) * P, icq * 512:(icq + 1) * 512],
                        in_=ysb[:],
                    )

                fill = [(icq, ob) for icq in range(3) for ob in range(4)]
                ytn_tiles[3] = ytn_pool.tile([P, NHL, 512], F16, name="ytn3", tag="ytn")
                for h in range(NHL):
                    st = bh(3, h)
                    for spl in (chain_fold, chain_bc, chain_mul):
                        icq, ob = fill.pop(0)
                        oproj_unit(icq, ob, splice=(lambda s=st, f=spl: f(s)))

            # ---------------- o_proj tail: dense, 4 PSUM banks ----------------
            with ExitStack() as phd:
                y_ps = phd.enter_context(tc.tile_pool(name="y_ps", bufs=4, space="PSUM"))
                ev2 = phd.enter_context(tc.tile_pool(name="ev2", bufs=4))
                for ic in range(ICH):
                    ytn = ytn_tiles[ic]
                    for ob in range(CT):
                        if (ic, ob) in done_obs:
                            continue
                        yp = y_ps.tile([P, 512], F32, name=f"yp{ic}_{ob}", tag="yp")
                        for h in range(NHL):
                            nc.tensor.matmul(
                                yp[:],
                                wo_sb[:, h, ob * P:(ob + 1) * P],
                                ytn[:, h, :],
                                start=(h == 0), stop=(h == NHL - 1),
                            )
                        ysb = ev2.tile([P, 512], F16, name=f"ysb{ic}_{ob}", tag="ysb")
                        nc.scalar.activation(
                            ysb[:], yp[:], mybir.ActivationFunctionType.Copy,
                            bias=0.0, scale=1.0 / DEN_SCALE,
                        )
                        nc.scalar.dma_start(
                            out=yt_out[ob * P:(ob + 1) * P, ic * 512:(ic + 1) * 512],
                            in_=ysb[:],
                        )

    nc.compile()
    _program_cache["nc"] = nc
    return nc


def _rope_tables():
    inv_freq = 1.0 / (10000.0 ** (np.arange(0, HD, 2, dtype=np.float32) / HD))
    pos = np.arange(S, dtype=np.float32)
    freqs = np.outer(pos, inv_freq).astype(np.float32)
    return np.cos(freqs).astype(np.float16), np.sin(freqs).astype(np.float16)


def _mask_tiles():
    # 1.0 strictly ABOVE the diagonal (j > i): multiplied by -30000*I inside
    # the score accumulation, these entries underflow to 0 after exp
    m = np.zeros((4, P, 512), dtype=np.float16)
    jj = np.arange(P)[:, None]
    ii = np.arange(512)[None, :]
    for t in range(4):
        m[t] = np.where(t * P + jj > ii, 1.0, 0.0)
    return m


def make_in_maps(x, Wq, Wk, Wv, Wo):
    x = np.asarray(x, dtype=np.float32)
    cos, sin = _rope_tables()
    maskt = _mask_tiles()
    wqT = np.ascontiguousarray(np.asarray(Wq, dtype=np.float32).T.astype(np.float16))
    wkT = np.ascontiguousarray(np.asarray(Wk, dtype=np.float32).T.astype(np.float16))
    wvT = np.ascontiguousarray(np.asarray(Wv, dtype=np.float32).T.astype(np.float16))
    woT = np.ascontiguousarray(np.asarray(Wo, dtype=np.float32).T.astype(np.float16))
    xts = [np.ascontiguousarray(x[b].T.astype(np.float16)) for b in range(2)]
    in_maps = []
    for c in range(NCORES):
        b, g = c // 4, c % 4
        sl = slice(g * GW, (g + 1) * GW)
        in_maps.append({
            "xt": xts[b],
            "wq": np.ascontiguousarray(wqT[:, sl]),
            "wk": np.ascontiguousarray(wkT[:, sl]),
            "wv": np.ascontiguousarray(wvT[:, sl]),
            "wo": np.ascontiguousarray(woT[sl, :]),
            "cos": cos,
            "sin": sin,
            "maskt": maskt,
        })
    return in_maps


def assemble_output(results):
    y = np.zeros((2, S, D), dtype=np.float32)
    for c in range(NCORES):
        b = c // 4
        y[b] += results[c]["yt_out"].T.astype(np.float32)
    return y


def kernel(x, Wq, Wk, Wv, Wo):
    nc = build_program()
    in_maps = make_in_maps(x, Wq, Wk, Wv, Wo)
    res = run_bass_kernel_spmd(nc, in_maps, core_ids=list(range(NCORES)))
    return assemble_output(res.results)


# revision 58
# speedup vs baseline: 1.0165x; 1.0165x over previous
"""Causal self-attention (QK-RMSNorm + RoPE) on 8 Trainium2 NeuronCores.

Problem: x[2,2048,2048], Wq/Wk/Wv/Wo [2048,2048], 16 heads, head_dim 128.

Sharding: core c handles batch b=c//4 and head group g=c%4 (4 heads,
model cols [512g:512g+512)).

Structure (v14):
  * QKV projection and attention are INTERLEAVED: attention chunk ic
    (dependency-limited: exp on ACT, masks/den on DVE) is spliced between
    the dense projection ib-passes of chunk ic+1, so the PE stays
    saturated and the HAM clock gate never re-throttles.  Q/K are
    RMS-normed + RoPE'd in row layout and PE-transposed into persistent
    SBUF tiles (no DRAM round trip).
  * Transposed scores: eT = exp(scale*kT_blk.T @ qT_chunk - 1); the causal
    mask is folded into the score accumulation as a second matmul
    (-30000*I @ above_diag_pattern), so masked entries underflow to 0 in
    the exp and no elementwise mask sits in the AV critical path.
  * Softmax denominator: DVE accumulates et tiles into [128,512] f32; a
    small matmul with (1/32) "ones" folds it across partitions, and a K=1
    matmul broadcasts the f16 reciprocal (32/den fits f16 normal range).
    The chain is spliced across the next projection pass boundaries so it
    never stalls the PE.  The 1/32 is divided back out at o_proj eviction.
  * o_proj: each core multiplies its yT head shard against its ROW slice
    of Wo.T, producing a full [2048 out, 2048 pos] f16 partial that the
    host sums across the 4 head groups.  No collective at all.  Runs as a
    dense tail phase with 4 PSUM banks.

Matmuls run with f16 operands (full PE rate).
"""

import math
from contextlib import ExitStack

import numpy as np

import concourse.bass as bass
import concourse.bacc as bacc
import concourse.tile as tile
from concourse import mybir
from concourse.bass_utils import run_bass_kernel_spmd
from concourse.masks import make_identity

P = 128
D = 2048
S = 2048
HD = 128              # head dim
NHL = 4               # heads per core
GW = NHL * HD         # 512, per-core width of head group
CT = D // P           # 16 contraction tiles
ICH = 4               # i-chunks of 512 positions
NCORES = 8
F32 = mybir.dt.float32
F16 = mybir.dt.float16
BF16 = mybir.dt.bfloat16
SCALE = 1.0 / math.sqrt(HD)
EPS = 1.1920928955078125e-07
# den spans roughly [1e-3, 1e5] over the causal rows; 32/den centers the
# reciprocal inside f16 normal range so the broadcast matmul can run in f16
DEN_SCALE = 32.0

_program_cache = {}


def build_program():
    if "nc" in _program_cache:
        return _program_cache["nc"]

    nc = bacc.Bacc("TRN2", target_bir_lowering=False, debug=False, num_devices=NCORES)
    # output stores round-robin over three trigger engines so the final
    # DMA drain overlaps instead of serializing on one queue
    dma_engines = [nc.sync, nc.scalar]

    xt_in = nc.dram_tensor("xt", [D, S], F16, kind="ExternalInput")
    wq_in = nc.dram_tensor("wq", [D, GW], F16, kind="ExternalInput")
    wk_in = nc.dram_tensor("wk", [D, GW], F16, kind="ExternalInput")
    wv_in = nc.dram_tensor("wv", [D, GW], F16, kind="ExternalInput")
    wo_in = nc.dram_tensor("wo", [GW, D], F16, kind="ExternalInput")
    cos_in = nc.dram_tensor("cos", [S, HD // 2], F16, kind="ExternalInput")
    sin_in = nc.dram_tensor("sin", [S, HD // 2], F16, kind="ExternalInput")
    mask_in = nc.dram_tensor("maskt", [4, P, 512], F16, kind="ExternalInput")
    yt_out = nc.dram_tensor("yt_out", [D, S], F16, kind="ExternalOutput")

    with tile.TileContext(nc) as tc:
        with ExitStack() as ctx:
            const = ctx.enter_context(tc.tile_pool(name="const", bufs=1))

            ident = const.tile([P, P], F16, name="ident")
            make_identity(nc, ident)
            negI = const.tile([P, P], F16, name="negI")
            nc.scalar.activation(
                negI[:], ident[:], mybir.ActivationFunctionType.Copy,
                bias=0.0, scale=-30000.0,
            )
            eps_t = const.tile([P, 1], F32, name="eps_t")
            nc.vector.memset(eps_t[:], EPS)
            neg1_t = const.tile([P, 1], F32, name="neg1_t")
            nc.vector.memset(neg1_t[:], -1.0)
            ones_f = const.tile([P, P], F32, name="ones_f")
            nc.vector.memset(ones_f[:], 1.0)
            onessc = const.tile([P, 2], F32, name="onessc")
            nc.vector.memset(onessc[:], 1.0 / DEN_SCALE)
            ones2 = const.tile([P, 2], BF16, name="ones2")
            nc.scalar.copy(ones2[:], onessc[:])
            ones_row = const.tile([1, P], F16, name="ones_row")
            nc.scalar.copy(ones_row[:], ones_f[0:1, :])
            invsc_t = const.tile([P, 1], F32, name="invsc_t")
            nc.vector.memset(invsc_t[:], 1.0 / DEN_SCALE)

            cos_sb = const.tile([P, CT, HD // 2], F16, name="cos_sb")
            nc.sync.dma_start(out=cos_sb[:], in_=cos_in.ap().rearrange("(a p) f -> p a f", p=P))
            sin_sb = const.tile([P, CT, HD // 2], F16, name="sin_sb")
            nc.sync.dma_start(out=sin_sb[:], in_=sin_in.ap().rearrange("(a p) f -> p a f", p=P))
            mask_sb = const.tile([P, 4, 512], F16, name="mask_sb")
            nc.sync.dma_start(out=mask_sb[:], in_=mask_in.ap().rearrange("t p f -> p t f"))

            # persistent tensors (live through the whole kernel)
            persist = ctx.enter_context(tc.tile_pool(name="persist", bufs=1))
            wq_sb = persist.tile([P, CT, GW], F16, name="wq_sb")
            wk_sb = persist.tile([P, CT, GW], F16, name="wk_sb")
            wv_sb = persist.tile([P, CT, GW], F16, name="wv_sb")
            wo_sb = persist.tile([P, NHL, D], F16, name="wo_sb")
            qt_sb = persist.tile([P, NHL, S], F16, name="qt_sb")
            kt_sb = persist.tile([P, NHL, S], F16, name="kt_sb")
            v_sb = persist.tile([P, CT, GW], F16, name="v_sb")

            # normalized attention outputs for all 4 chunks (consumed by the
            # o_proj tail phase)
            ytn_pool = ctx.enter_context(tc.tile_pool(name="ytn_pool", bufs=4))
            ytn_tiles = {}

            # ---------------- merged phase: QKV projection + attention ------
            with ExitStack() as pha:
                xt_pool = pha.enter_context(tc.tile_pool(name="xt_pool", bufs=2))
                proj_ps = pha.enter_context(tc.tile_pool(name="proj_ps", bufs=2, space="PSUM"))
                tp_ps = pha.enter_context(tc.tile_pool(name="tp_ps", bufs=1, space="PSUM"))
                rope = pha.enter_context(tc.tile_pool(name="rope", bufs=3))
                stat = pha.enter_context(tc.tile_pool(name="stat", bufs=3))
                s_ps = pha.enter_context(tc.tile_pool(name="s_ps", bufs=3, space="PSUM"))
                acc_ps = pha.enter_context(tc.tile_pool(name="acc_ps", bufs=1, space="PSUM"))
                dbc_ps = pha.enter_context(tc.tile_pool(name="dbc_ps", bufs=1, space="PSUM"))
                et_pool = pha.enter_context(tc.tile_pool(name="et_pool", bufs=6))
                den_pool = pha.enter_context(tc.tile_pool(name="den_pool", bufs=2))
                bsmall = pha.enter_context(tc.tile_pool(name="bsmall", bufs=2))

                # dummy matmuls bridge the initial weight/x DMA wait so the
                # HAM clock gate is already released when real work arrives
                warm = dbc_ps.tile([P, 512], F32, name="warm", tag="dbc")
                NWARM = 140
                for i in range(NWARM):
                    nc.tensor.matmul(
                        warm[:, 0:P], ident[:], ident[:],
                        start=(i == 0), stop=(i == NWARM - 1),
                    )
                warm_out = const.tile([P, 1], F32, name="warm_out")
                nc.scalar.copy(warm_out[:], warm[:, 0:1])

                xt_tiles = {}

                def a_dmas(ch):
                    xt_ch = xt_pool.tile([P, CT, 512], F16, name=f"xt_ch{ch}", tag="xt")
                    xt_tiles[ch] = xt_ch
                    for ct in range(CT):
                        if ch == 0:
                            nc.sync.dma_start(out=wq_sb[:, ct, :], in_=wq_in[ct * P:(ct + 1) * P, :])
                        nc.sync.dma_start(
                            out=xt_ch[:, ct, :],
                            in_=xt_in[ct * P:(ct + 1) * P, ch * 512:(ch + 1) * 512],
                        )
                    if ch == 0:
                        for ct in range(CT):
                            nc.sync.dma_start(out=wk_sb[:, ct, :], in_=wk_in[ct * P:(ct + 1) * P, :])
                        for ct in range(CT):
                            nc.sync.dma_start(out=wv_sb[:, ct, :], in_=wv_in[ct * P:(ct + 1) * P, :])
                    if ch == 1:
                        for hb in range(NHL):
                            nc.sync.dma_start(
                                out=wo_sb[:, hb, :], in_=wo_in[hb * P:(hb + 1) * P, :]
                            )

                def norm_rope(ps_, ibg, mat):
                    qs = rope.tile([P, GW], F16, name=f"qs{mat}{ibg}", tag="qs")
                    nc.scalar.copy(qs[:], ps_[:])
                    sq = rope.tile([P, GW], F16, name=f"sq{mat}{ibg}", tag="sq")
                    nc.vector.tensor_mul(sq[:], qs[:], qs[:])
                    rstd = stat.tile([P, NHL], F32, name=f"rstd{mat}{ibg}", tag="rstd")
                    nc.vector.reduce_sum(
                        rstd[:],
                        sq[:].rearrange("p (h d) -> p h d", h=NHL),
                        axis=mybir.AxisListType.X,
                    )
                    nc.scalar.activation(
                        rstd[:], rstd[:],
                        mybir.ActivationFunctionType.Sqrt,
                        bias=eps_t[:], scale=1.0 / HD,
                    )
                    nc.vector.reciprocal(rstd[:], rstd[:])

                    q3 = qs[:].rearrange("p (h d) -> p h d", h=NHL)
                    qr = rope.tile([P, GW], F16, name=f"qr{mat}{ibg}", tag="qr")
                    qr3 = qr[:].rearrange("p (h d) -> p h d", h=NHL)
                    tmp = rope.tile([P, NHL, HD // 2], F16, name=f"tmp{mat}{ibg}", tag="tmp")
                    cosB = cos_sb[:, ibg:ibg + 1, :].broadcast_to((P, NHL, HD // 2))
                    sinB = sin_sb[:, ibg:ibg + 1, :].broadcast_to((P, NHL, HD // 2))
                    h1 = q3[:, :, 0:HD // 2]
                    h2 = q3[:, :, HD // 2:HD]
                    # r1 = q1*cos + q2*sin ; r2 = q2*cos - q1*sin
                    nc.vector.tensor_mul(qr3[:, :, 0:HD // 2], h1, cosB)
                    nc.vector.tensor_mul(tmp[:], h2, sinB)
                    nc.vector.tensor_add(qr3[:, :, 0:HD // 2], qr3[:, :, 0:HD // 2], tmp[:])
                    nc.vector.tensor_mul(qr3[:, :, HD // 2:HD], h2, cosB)
                    nc.vector.tensor_mul(tmp[:], h1, sinB)
                    nc.vector.tensor_sub(
                        qr3[:, :, HD // 2:HD], qr3[:, :, HD // 2:HD], tmp[:]
                    )
                    for h in range(NHL):
                        nc.vector.tensor_scalar_mul(
                            qr[:, h * HD:(h + 1) * HD],
                            qr[:, h * HD:(h + 1) * HD],
                            rstd[:, h:h + 1],
                        )
                    return qr

                pend_tp = []  # transposes deferred ~2 units behind the DVE chain

                def flush_tp():
                    qr, ibg, dst_t, mat = pend_tp.pop(0)
                    # all 4 heads transpose into one PSUM tile back-to-back,
                    # then a single strided copy writes the persistent tensor
                    tp = tp_ps.tile([P, NHL, P], F16, name=f"tp{mat}{ibg}", tag="tp")
                    for h in range(NHL):
                        nc.tensor.transpose(tp[:, h, :], qr[:, h * HD:(h + 1) * HD], ident[:])
                    nc.scalar.copy(dst_t[:, :, ibg * P:(ibg + 1) * P], tp[:])

                MATS = (("q", wq_sb, qt_sb), ("k", wk_sb, kt_sb), ("v", wv_sb, None))

                def a_ib_unit(ch, mi, ib, splice=None):
                    """One (matrix, i-block) projection pass: 16 dense matmuls
                    (+ spliced den-chain PE ops), then the norm/rope chain."""
                    mat, wsb, dst_t = MATS[mi]
                    ibg = ch * 4 + ib
                    xt_ch = xt_tiles[ch]
                    ps = proj_ps.tile([P, GW], F32, name=f"ps{mat}{ibg}", tag="proj")
                    for ct in range(CT):
                        nc.tensor.matmul(
                            ps[:],
                            xt_ch[:, ct, ib * P:(ib + 1) * P],
                            wsb[:, ct, :],
                            start=(ct == 0),
                            stop=(ct == CT - 1),
                        )
                    if splice is not None:
                        splice()
                    if mat == "v":
                        nc.scalar.copy(v_sb[:, ibg, :], ps[:])
                    else:
                        if len(pend_tp) >= 2:
                            flush_tp()
                        qr = norm_rope(ps, ibg, mat)
                        pend_tp.append((qr, ibg, dst_t, mat))

                # ---- attention pieces ----
                def bh(ic, h):
                    """One head's attention for i-chunk ic; returns the den
                    chain state (finished by spliced chain steps)."""
                    njb = 4 * ic + 4
                    ytn = ytn_tiles[ic]
                    ytp = acc_ps.tile([P, 512], F32, name=f"yt{h}_{ic}", tag="yt")
                    dacc = den_pool.tile([P, 512], F32, name=f"den{h}_{ic}", tag="den")
                    ets = {}

                    def score(jb):
                        t = jb - 4 * ic
                        sp = s_ps.tile([P, 512], F32, name=f"s{h}_{ic}_{jb}", tag="s")
                        nc.tensor.matmul(
                            sp[:],
                            kt_sb[:, h, jb * P:(jb + 1) * P],
                            qt_sb[:, h, ic * 512:(ic + 1) * 512],
                            start=True, stop=(t < 0),
                        )
                        if t >= 0:
                            nc.tensor.matmul(
                                sp[:], negI[:], mask_sb[:, t, :],
                                start=False, stop=True,
                            )
                        et = et_pool.tile([P, 512], F16, name=f"et{h}_{ic}_{jb}", tag="et")
                        nc.scalar.activation(
                            et[:], sp[:],
                            mybir.ActivationFunctionType.Exp,
                            bias=neg1_t[:], scale=SCALE,
                        )
                        ets[jb] = et

                    def consume(jb):
                        et = ets.pop(jb)
                        nc.tensor.matmul(
                            ytp[:],
                            v_sb[:, jb, h * HD:(h + 1) * HD],
                            et[:],
                            start=(jb == 0), stop=(jb == njb - 1),
                        )
                        if jb == 0:
                            nc.vector.tensor_copy(dacc[:], et[:])
                        else:
                            nc.vector.tensor_add(dacc[:], dacc[:], et[:])

                    for jb in range(min(3, njb)):
                        score(jb)
                    for jb in range(njb):
                        if jb + 3 < njb:
                            score(jb + 3)
                        consume(jb)

                    # cast for the den fold is issued now (ACT digests it
                    # while the next dense pass runs)
                    dacc_b = den_pool.tile([P, 512], BF16, name=f"daccb{h}_{ic}", tag="daccb")
                    nc.scalar.copy(dacc_b[:], dacc[:])
                    return {"ytp": ytp, "dacc_b": dacc_b, "ytn": ytn, "h": h, "ic": ic}

                def chain_fold(st):
                    # fold den across partitions; reciprocal lands in f16
                    dbc = dbc_ps.tile([P, 512], F32, name=f"dbc{st['h']}_{st['ic']}", tag="dbc")
                    st["dbc"] = dbc
                    nc.tensor.matmul(dbc[0:2, :], ones2[:], st["dacc_b"][:], start=True, stop=True)
                    rden = bsmall.tile([1, 512], F32, name=f"rd{st['h']}_{st['ic']}", tag="rden")
                    nc.vector.reciprocal(rden[:], dbc[0:1, :])
                    rden_r = bsmall.tile([1, 512], F16, name=f"rdr{st['h']}_{st['ic']}", tag="rdenr")
                    nc.scalar.copy(rden_r[:], rden[:])
                    st["rden_r"] = rden_r

                def chain_bc(st):
                    nc.tensor.matmul(st["dbc"][:], ones_row[:], st["rden_r"][:], start=True, stop=True)

                def chain_mul(st):
                    bc_sb = bsmall.tile([P, 512], F32, name=f"bcs{st['h']}_{st['ic']}", tag="bcs")
                    nc.vector.tensor_copy(bc_sb[:], st["dbc"][:])
                    nc.vector.tensor_mul(st["ytn"][:, st["h"], :], st["ytp"][:], bc_sb[:])

                # ---- schedule ----
                a_dmas(0)
                a_dmas(1)
                for mi in range(3):
                    for ib in range(4):
                        a_ib_unit(0, mi, ib)
                while pend_tp:
                    # chunk 0's last transposes must land before its
                    # attention reads qt/kt below
                    flush_tp()

                for ic in range(ICH - 1):
                    ch = ic + 1
                    if ch + 1 < ICH:
                        a_dmas(ch + 1)
                    ytn_tiles[ic] = ytn_pool.tile([P, NHL, 512], F16,
                                                  name=f"ytn{ic}", tag="ytn")
                    units = [(mi, ib) for mi in range(3) for ib in range(4)]
                    for h in range(NHL):
                        st = bh(ic, h)
                        for step, spl in ((0, chain_fold), (1, chain_bc), (2, chain_mul)):
                            mi, ib = units.pop(0)
                            a_ib_unit(ch, mi, ib,
                                      splice=(lambda s=st, f=spl: f(s)))
                    while pend_tp:
                        flush_tp()

                # last chunk's attention: o_proj blocks of the finished
                # chunks 0-2 serve as the dense PE filler (their PSUM bank is
                # borrowed from the den pool; evictions go through DVE since
                # ACT is saturated with this chunk's exps)
                done_obs = set()

                def oproj_unit(icq, ob, splice=None):
                    # borrows the (now idle) projection pool's PSUM banks
                    done_obs.add((icq, ob))
                    ytnq = ytn_tiles[icq]
                    yp = proj_ps.tile([P, 512], F32, name=f"ypf{icq}_{ob}", tag="proj")
                    for hh in range(NHL):
                        nc.tensor.matmul(
                            yp[:],
                            wo_sb[:, hh, ob * P:(ob + 1) * P],
                            ytnq[:, hh, :],
                            start=(hh == 0), stop=(hh == NHL - 1),
                        )
                    if splice is not None:
                        splice()
                    ysb = bsmall.tile([P, 512], F16, name=f"ysbf{icq}_{ob}", tag="ysbf")
                    nc.vector.tensor_scalar_mul(ysb[:], yp[:], invsc_t[:])
                    dma_engines[(icq * CT + ob) % 2].dma_start(
                        out=yt_out[ob * P:(ob + 1) * P, icq * 512:(icq + 1) * 512],
                        in_=ysb[:],
                    )

                fill = [(icq, ob) for icq in range(3) for ob in range(4)]
                ytn_tiles[3] = ytn_pool.tile([P, NHL, 512], F16, name="ytn3", tag="ytn")
                for h in range(NHL):
                    st = bh(3, h)
                    for spl in (chain_fold, chain_bc, chain_mul):
                        icq, ob = fill.pop(0)
                        oproj_unit(icq, ob, splice=(lambda s=st, f=spl: f(s)))

            # ---------------- o_proj tail: dense, 4 PSUM banks ----------------
            with ExitStack() as phd:
                y_ps = phd.enter_context(tc.tile_pool(name="y_ps", bufs=4, space="PSUM"))
                ev2 = phd.enter_context(tc.tile_pool(name="ev2", bufs=4))
                for ic in range(ICH):
                    ytn = ytn_tiles[ic]
                    for ob in range(CT):
                        if (ic, ob) in done_obs:
                            continue
                        yp = y_ps.tile([P, 512], F32, name=f"yp{ic}_{ob}", tag="yp")
                        for h in range(NHL):
                            nc.tensor.matmul(
                                yp[:],
                                wo_sb[:, h, ob * P:(ob + 1) * P],
                                ytn[:, h, :],
                                start=(h == 0), stop=(h == NHL - 1),
                            )
                        ysb = ev2.tile([P, 512], F16, name=f"ysb{ic}_{ob}", tag="ysb")
                        nc.scalar.activation(
                            ysb[:], yp[:], mybir.ActivationFunctionType.Copy,
                            bias=0.0, scale=1.0 / DEN_SCALE,
                        )
                        dma_engines[(ic * CT + ob) % 2].dma_start(
                            out=yt_out[ob * P:(ob + 1) * P, ic * 512:(ic + 1) * 512],
                            in_=ysb[:],
                        )

    nc.compile()
    _program_cache["nc"] = nc
    return nc


def _rope_tables():
    inv_freq = 1.0 / (10000.0 ** (np.arange(0, HD, 2, dtype=np.float32) / HD))
    pos = np.arange(S, dtype=np.float32)
    freqs = np.outer(pos, inv_freq).astype(np.float32)
    return np.cos(freqs).astype(np.float16), np.sin(freqs).astype(np.float16)


def _mask_tiles():
    # 1.0 strictly ABOVE the diagonal (j > i): multiplied by -30000*I inside
    # the score accumulation, these entries underflow to 0 after exp
    m = np.zeros((4, P, 512), dtype=np.float16)
    jj = np.arange(P)[:, None]
    ii = np.arange(512)[None, :]
    for t in range(4):
        m[t] = np.where(t * P + jj > ii, 1.0, 0.0)
    return m


def make_in_maps(x, Wq, Wk, Wv, Wo):
    x = np.asarray(x, dtype=np.float32)
    cos, sin = _rope_tables()
    maskt = _mask_tiles()
    wqT = np.ascontiguousarray(np.asarray(Wq, dtype=np.float32).T.astype(np.float16))
    wkT = np.ascontiguousarray(np.asarray(Wk, dtype=np.float32).T.astype(np.float16))
    wvT = np.ascontiguousarray(np.asarray(Wv, dtype=np.float32).T.astype(np.float16))
    woT = np.ascontiguousarray(np.asarray(Wo, dtype=np.float32).T.astype(np.float16))
    xts = [np.ascontiguousarray(x[b].T.astype(np.float16)) for b in range(2)]
    in_maps = []
    for c in range(NCORES):
        b, g = c // 4, c % 4
        sl = slice(g * GW, (g + 1) * GW)
        in_maps.append({
            "xt": xts[b],
            "wq": np.ascontiguousarray(wqT[:, sl]),
            "wk": np.ascontiguousarray(wkT[:, sl]),
            "wv": np.ascontiguousarray(wvT[:, sl]),
            "wo": np.ascontiguousarray(woT[sl, :]),
            "cos": cos,
            "sin": sin,
            "maskt": maskt,
        })
    return in_maps


def assemble_output(results):
    y = np.zeros((2, S, D), dtype=np.float32)
    for c in range(NCORES):
        b = c // 4
        y[b] += results[c]["yt_out"].T.astype(np.float32)
    return y


def kernel(x, Wq, Wk, Wv, Wo):
    nc = build_program()
    in_maps = make_in_maps(x, Wq, Wk, Wv, Wo)
    res = run_bass_kernel_spmd(nc, in_maps, core_ids=list(range(NCORES)))
    return assemble_output(res.results)
